# revision 4
# baseline (speedup 1.0000x reference)
"""MiniBatchDiscrimination kernel, v2: symmetric-pair sharding.

Each unordered block-pair {a,b} of the 8x8 grid of 64x64 pairwise blocks is
computed exactly once: core c processes its row block c against column
blocks [c, c+1, c+2, c+3] (mod 8) plus, for c<4, block c+4 (distance-4
pairs).  Cores 4-7 get a 5th "poison" column block (xT columns = 1e4) whose
distances are astronomically large, so exp() contributes exactly 0 - this
keeps the kernel SPMD-identical across cores at FD=320.

Per pair-block, both marginals are extracted:
  - row-sums (for the core's own rows) via ACT Exp accum_out,
  - column-sums (partials for the partner block's rows) via PE matmuls of
    the exp tiles against a stacked-identity selector, accumulated in PSUM
    chunks and drained into an SBUF fp32 accumulator.
The host adds the partials into the partner row blocks.

Sync-budget notes (walrus S3D3 encoding): TensorScalarPtr and Activation
instructions only fit ONE sync-wait command, and Tile emits non-reduced
self-waits plus cross-engine WAR waits on pool-slot reuse.  Hence:
  - all elementwise |.| tiles run on DVE only (1 self-wait each),
  - a 1-element DVE copy from an old PSUM bank each iteration refreshes
    DVE's observed PE clock, so slot-reuse PE waits are pre-satisfied and
    never emitted (workV bufs span 4 iterations),
  - Exp output tiles are never reused (fresh slot per use), so Exp carries
    only its PE wait.
"""

import numpy as np
import ml_dtypes
from contextlib import ExitStack

BATCH, IN_FEAT, OUT_FEAT, KERNEL_DIM = 512, 512, 64, 32
N_CORES = 8
ROWB = BATCH // N_CORES          # 64 rows of i per core
OK = OUT_FEAT * KERNEL_DIM       # 2048 flattened (o,k)
NT = OK // 128                   # 16 partition-tiles of (o,k)
NBLK = 5                         # column blocks per core
FD = NBLK * 64                   # 320
POISON = 1.0e4

CHUNK = 8                        # i's per colsum PSUM chunk
SELW = OUT_FEAT
ADV_BUFS = 64                    # 4 iterations of elementwise tiles

_cache = {}


def _build_nc(split_waits=True):
    import concourse.bass as bass
    import concourse.mybir as mybir
    import concourse.tile as tile
    from concourse.tile_rust import add_dep_helper

    dt = mybir.dt
    AF = mybir.ActivationFunctionType
    OP = mybir.AluOpType

    nc = bass.Bass("TRN2", target_bir_lowering=False, debug=False,
                   num_devices=N_CORES)

    xT_d = nc.dram_tensor("xT", [IN_FEAT, FD], dt.bfloat16, kind="ExternalInput")
    T_d = nc.dram_tensor("Tm", [IN_FEAT, OK], dt.bfloat16, kind="ExternalInput")
    sel_d = nc.dram_tensor("sel", [128, NT * SELW], dt.bfloat16,
                           kind="ExternalInput")
    sel2_d = nc.dram_tensor("sel2", [128, OUT_FEAT], dt.bfloat16,
                            kind="ExternalInput")
    selS_d = nc.dram_tensor("selS", [128, NT * SELW], dt.bfloat16,
                            kind="ExternalInput")
    dneg_d = nc.dram_tensor("dneg", [OUT_FEAT, OUT_FEAT], dt.bfloat16,
                            kind="ExternalInput")
    rows_d = nc.dram_tensor("rowS", [OUT_FEAT, ROWB], dt.float32,
                            kind="ExternalOutput")
    acc_d = nc.dram_tensor("accS", [OUT_FEAT, FD], dt.float32,
                           kind="ExternalOutput")

    with tile.TileContext(nc) as tc, ExitStack() as ctx:
        const = ctx.enter_context(tc.tile_pool(name="const", bufs=1))
        mtp = ctx.enter_context(tc.tile_pool(name="mt", bufs=NT))
        psA = ctx.enter_context(
            tc.tile_pool(name="psA", bufs=1, space=bass.MemorySpace.PSUM))
        psD = ctx.enter_context(
            tc.tile_pool(name="psD", bufs=3, space=bass.MemorySpace.PSUM))
        psC = ctx.enter_context(
            tc.tile_pool(name="psC", bufs=1, space=bass.MemorySpace.PSUM))
        # dedicated PSUM bank for the PE "clock": a tiny matmul at the end of
        # each iteration writes here, and the DVE fence reads it two
        # iterations later.  No ACT instruction ever touches this bank, so
        # the fence carries exactly one (PE) wait.
        psK = ctx.enter_context(
            tc.tile_pool(name="psK", bufs=2, space=bass.MemorySpace.PSUM))
        workV = ctx.enter_context(tc.tile_pool(name="workV", bufs=ADV_BUFS))
        ep = ctx.enter_context(
            tc.tile_pool(name="e", bufs=ROWB // 2))  # fresh tile per use
        fences = ctx.enter_context(
            tc.tile_pool(name="fences", bufs=ROWB))  # fresh tile per use

        Tsb = []
        for kc in range(4):
            t_ = const.tile([128, OK], dt.bfloat16, tag=f"T{kc}")
            nc.sync.dma_start(t_[:], T_d[kc * 128:(kc + 1) * 128, :])
            Tsb.append(t_)
        xTsb = []
        for kc in range(4):
            t_ = const.tile([128, FD], dt.bfloat16, tag=f"x{kc}")
            nc.sync.dma_start(t_[:], xT_d[kc * 128:(kc + 1) * 128, :])
            xTsb.append(t_)
        sel = const.tile([128, NT * SELW], dt.bfloat16, tag="sel")
        nc.sync.dma_start(sel[:], sel_d[:])
        sel2 = const.tile([128, OUT_FEAT], dt.bfloat16, tag="sel2")
        nc.sync.dma_start(sel2[:], sel2_d[:])
        selS = const.tile([128, NT * SELW], dt.bfloat16, tag="selS")
        nc.sync.dma_start(selS[:], selS_d[:])
        dneg = const.tile([OUT_FEAT, OUT_FEAT], dt.bfloat16, tag="dneg")
        nc.sync.dma_start(dneg[:], dneg_d[:])
        mcol = const.tile([128, NT * ROWB], dt.float32, tag="mcol")
        rowS = const.tile([OUT_FEAT, ROWB], dt.float32, tag="rowS")
        accS = const.tile([OUT_FEAT, FD], dt.float32, tag="accS")
        nc.vector.memset(accS[:], 0.0)

        # Mt tiles: Mt[(o,k), j], tile t holds o in [4t, 4t+4), all k
        mts = []
        for t in range(NT):
            ps = psA.tile([128, FD], dt.float32)
            for kc in range(4):
                nc.tensor.matmul(ps[:],
                                 Tsb[kc][:, t * 128:(t + 1) * 128],
                                 xTsb[kc][:],
                                 start=(kc == 0), stop=(kc == 3))
            mt_t = mtp.tile([128, FD], dt.bfloat16, tag="mt")
            nc.vector.tensor_copy(mt_t[:], ps[:])
            # scalar columns: the *rounded* bf16 values recast to fp32 so the
            # diagonal difference is exactly zero
            nc.vector.tensor_copy(mcol[:, t * ROWB:(t + 1) * ROWB],
                                  mt_t[:, 0:ROWB])
            mts.append(mt_t)

        # S[o, j] = sum_k Mt[(o,k), j]; kept in bf16 so the Exp bias (the
        # bf16-rounded S column) cancels the matmul term exactly on the
        # diagonal: D_ii = 2*0 + S_i - S_i = 0.
        psS = psA.tile([OUT_FEAT, FD], dt.float32, tag="psS")
        for t in range(NT):
            nc.tensor.matmul(psS[:], selS[:, t * SELW:(t + 1) * SELW],
                             mts[t][:], start=(t == 0), stop=(t == NT - 1))
        S_bf = const.tile([OUT_FEAT, FD], dt.bfloat16, tag="S_bf")
        nc.vector.tensor_copy(S_bf[:], psS[:])
        Sneg = const.tile([OUT_FEAT, ROWB], dt.float32, tag="Sneg")
        nc.vector.tensor_scalar(Sneg[:], S_bf[:, 0:ROWB], -1.0, None,
                                op0=OP.mult)
        # warm up ACT's observed DVE clock so the first Exp (whose bias is
        # the DVE-written Sneg) does not need a second sync wait
        warmA = const.tile([1, 1], dt.float32, tag="warmA")
        nc.scalar.copy(warmA[:], Sneg[0:1, 0:1])

        clocks = []
        n_chunks = ROWB // CHUNK
        for ch in range(n_chunks):
            e_tiles = []
            for ii in range(CHUNK):
                i = ch * CHUNK + ii
                psd = psD.tile([SELW, FD], dt.float32)
                ads = [workV.tile([128, FD], dt.bfloat16, tag="adV",
                                  name=f"ad_{i}_{t}")
                       for t in range(NT)]
                fence_inst = None
                if i >= 2:
                    # refresh DVE's observed PE clock: a 1-element copy from
                    # an old PSUM bank into a fresh scratch tile carries the
                    # PE wait (its only wait), so the tensor_scalar ops'
                    # slot-reuse never needs a cross-engine wait
                    fence = fences.tile([1, 1], dt.float32, tag="fence",
                                        name=f"fence_{i}")
                    fence_inst = nc.vector.tensor_copy(
                        fence[:], clocks[i - 2][:])
                for t in range(NT):
                    sc = mcol[:, t * ROWB + i: t * ROWB + i + 1]
                    ts_inst = nc.vector.tensor_scalar(
                        ads[t][:], mts[t][:], sc, 0.0,
                        op0=OP.subtract, op1=OP.max)
                    if fence_inst is not None:
                        # ordering-only edge: keeps the fence scheduled ahead
                        # of this iteration's elementwise ops
                        add_dep_helper(ts_inst.ins, fence_inst.ins,
                                       sync=False,
                                       reason="PE-clock fence ordering")
                    nc.tensor.matmul(psd[:],
                                     sel[:, t * SELW:(t + 1) * SELW],
                                     ads[t][:],
                                     start=(t == 0), stop=False)
                # psd holds 2*sum_k relu(Mt - Mt_i); subtract S_j here and
                # add S_i via the Exp bias: D = 2*sum relu(d) + S_i - S_j
                nc.tensor.matmul(psd[:], dneg[:], S_bf[:],
                                 start=False, stop=True)
                clk = psK.tile([1, 1], dt.float32, tag="clk", name=f"clk_{i}")
                nc.tensor.matmul(clk[:], sel[:, 0:1], ads[NT - 1][:, 0:1],
                                 start=True, stop=True)
                clocks.append(clk)
                # exp tiles packed two i's per [128, FD]: even i in
                # partitions 0:64, odd i in 64:128
                if ii % 2 == 0:
                    e_t = ep.tile([128, FD], dt.bfloat16, tag="e")
                    e_tiles.append(e_t)
                half = e_tiles[-1][(ii % 2) * OUT_FEAT:
                                   (ii % 2 + 1) * OUT_FEAT, :]
                nc.scalar.activation(half, psd[:], AF.Exp, scale=-1.0,
                                     bias=Sneg[:, i:i + 1],
                                     accum_out=rowS[:, i:i + 1])
            # column-sum partials for this chunk: PE reduce over the packed
            # i-partition pairs, then fold into the fp32 SBUF accumulator
            psc = psC.tile([OUT_FEAT, FD], dt.float32)
            for m, e_t in enumerate(e_tiles):
                nc.tensor.matmul(psc[:], sel2[:], e_t[:],
                                 start=(m == 0), stop=(m == len(e_tiles) - 1))
            nc.vector.tensor_add(accS[:], accS[:], psc[:])

        # outputs go out on the SW-DGE queues (gpsimd): the HW-DGE queues
        # carried the input loads, and a shared queue would add a second
        # sync-wait command that the DMA pseudo-instruction cannot encode
        nc.gpsimd.dma_start(rows_d[:], rowS[:])
        nc.gpsimd.dma_start(acc_d[:], accS[:])

    if split_waits:
        _split_multiwaits(nc, mybir)
    return nc


def _split_multiwaits(nc, mybir):
    """Walrus on this toolchain encodes at most ONE sync-wait command per
    instruction.  Split any instruction with more waits (in practice only
    the framework's kernel-tail drain) into a chain of single-wait Drain
    carriers on the same engine, inserted immediately before it."""
    n = 0
    for fn in nc.m.functions:
        for bb in fn.blocks:
            new_insts = []
            for inst in bb.instructions:
                si = getattr(inst, "sync_info", None)
                if si is not None and si.on_wait and len(si.on_wait) > 1:
                    waits = list(si.on_wait)
                    for w in waits[:-1]:
                        carrier = mybir.InstDrain(
                            name=f"splitw_{n}", engine=inst.engine,
                            ins=[], outs=[],
                            sync_info=mybir.SyncInfo(on_wait=[w],
                                                     on_update=[]))
                        new_insts.append(carrier)
                        n += 1
                    inst.sync_info = mybir.SyncInfo(
                        on_wait=[waits[-1]], on_update=list(si.on_update))
                new_insts.append(inst)
            if n:
                bb.instructions = new_insts


def _sel_host(value):
    sel = np.zeros((128, NT * SELW), dtype=np.float32)
    for t in range(NT):
        for g in range(4):
            sel[32 * g:32 * (g + 1), t * SELW + 4 * t + g] = value
    return sel.astype(ml_dtypes.bfloat16)


def _sel2_host():
    s = np.zeros((128, OUT_FEAT), dtype=np.float32)
    s[:OUT_FEAT, :] = np.eye(OUT_FEAT)
    s[OUT_FEAT:, :] = np.eye(OUT_FEAT)
    return s.astype(ml_dtypes.bfloat16)


def _block_order(c):
    """Column blocks for core c; None marks the poison block."""
    if c < 4:
        return [c, c + 1, c + 2, c + 3, c + 4]
    return [c, (c + 1) % 8, (c + 2) % 8, (c + 3) % 8, None]


def _in_maps(x, T):
    bf16 = ml_dtypes.bfloat16
    Tb = np.ascontiguousarray(T.reshape(IN_FEAT, OK)).astype(bf16)
    selb = _sel_host(2.0)
    selSb = _sel_host(1.0)
    sel2b = _sel2_host()
    dnegb = (-np.eye(OUT_FEAT, dtype=np.float32)).astype(bf16)
    xT = np.ascontiguousarray(x.T)
    maps = []
    for c in range(N_CORES):
        xTc = np.empty((IN_FEAT, FD), dtype=np.float32)
        for pos, b in enumerate(_block_order(c)):
            if b is None:
                xTc[:, 64 * pos:64 * (pos + 1)] = POISON
            else:
                xTc[:, 64 * pos:64 * (pos + 1)] = xT[:, 64 * b:64 * (b + 1)]
        maps.append({"xT": xTc.astype(bf16), "Tm": Tb, "sel": selb,
                     "selS": selSb, "sel2": sel2b, "dneg": dnegb})
    return maps


def kernel(x, T):
    from concourse import bass_utils

    x = np.asarray(x, dtype=np.float32)
    T = np.asarray(T, dtype=np.float32)

    if "nc" not in _cache:
        _cache["nc"] = _build_nc()
    nc = _cache["nc"]

    res = bass_utils.run_bass_kernel_spmd(
        nc, _in_maps(x, T), core_ids=list(range(N_CORES)))

    mbd = np.zeros((BATCH, OUT_FEAT), dtype=np.float32)
    for c in range(N_CORES):
        rs = np.asarray(res.results[c]["rowS"], dtype=np.float32)  # [o, i]
        mbd[64 * c:64 * (c + 1), :] += rs.T
        acc = np.asarray(res.results[c]["accS"], dtype=np.float32)  # [o, j]
        for pos, b in enumerate(_block_order(c)):
            if pos == 0 or b is None:
                continue  # own diag block is fully in rowsums; poison dropped
            mbd[64 * b:64 * (b + 1), :] += acc[:, 64 * pos:64 * (pos + 1)].T
    mbd -= 1.0
    return np.concatenate([x, mbd], axis=1)
